# revision 1
# baseline (speedup 1.0000x reference)
"""EnhancedContrastiveLoss on 8 Trainium2 NeuronCores (Bass/Tile).

Strategy
--------
Host side (layout only, no FLOPs):
  * sort samples by label; shard rows 1024/core; per core rotate the
    column order by (row0-128) so every core sees its own rows' class
    neighborhoods at the same local column positions (SPMD-constant
    addressing), and transpose to [D, B] for the matmul operands.
Device side (per core, all FLOPs):
  * column norms via square+ones-matmul, inv = 1/max(sqrt(n2),1e-12)
  * normalize operands (PE broadcast of inv + fused scale on DVE)
  * sim row-tiles [128, 8192] = etn_rows^T @ etn (fp32r matmuls)
  * E = exp(sim/T) on ACT with fused row-sum accumulation
  * label-band ops (positives live in a 384-wide diagonal band after
    sorting): pos_sum, pos_count, self term, pos_max; mask the band
    out of E, then hardware top-8 (InstMax) gives the top-3 negatives
  * per-row losses from the stats; output [128, 32] partial sums
Host side: combine 8 cores' partials into the 3 scalar losses.
"""

import numpy as np
from contextlib import ExitStack

import concourse.bass as bass
import concourse.mybir as mybir
from concourse import bacc, tile
from concourse.bass_utils import run_bass_kernel_spmd

F32 = mybir.dt.float32
F32R = mybir.dt.float32r
AF = mybir.ActivationFunctionType
ALU = mybir.AluOpType
AX = mybir.AxisListType

B = 8192
D = 256
NC = 8
RPC = B // NC          # rows per core
NT = RPC // 128        # row tiles per core (8)
CH = 512               # matmul N-chunk
NCH = B // CH          # 16
KT = D // 128          # K tiles (2)
BAND = 384
TEMP = 0.07
MARGIN = 0.2
INVT = 1.0 / TEMP
NEG_BIG = -1.0e30

_CACHE = {}


def _build_program():
    if "nc" in _CACHE:
        return _CACHE["nc"]
    nc = bacc.Bacc(
        "TRN2", target_bir_lowering=False, debug=False, num_devices=NC
    )
    et_d = nc.dram_tensor("et", [D, B], F32, kind="ExternalInput").ap()
    lab_d = nc.dram_tensor("labf", [1, B], F32, kind="ExternalInput").ap()
    eye_d = nc.dram_tensor("eye", [128, BAND], F32, kind="ExternalInput").ap()
    out_d = nc.dram_tensor("out", [128, 32], F32, kind="ExternalOutput").ap()

    with tile.TileContext(nc) as tc:
        with ExitStack() as ctx:
            _body(ctx, tc, et_d, lab_d, eye_d, out_d)

    nc.finalize()
    _CACHE["nc"] = nc
    return nc


def _body(ctx, tc, et_d, lab_d, eye_d, out_d):
    nc = tc.nc
    r32 = lambda ap: ap.bitcast(F32R)

    singles = ctx.enter_context(tc.tile_pool(name="singles", bufs=1))
    etnpool = ctx.enter_context(tc.tile_pool(name="etn", bufs=2))
    bigpool = ctx.enter_context(tc.tile_pool(name="big", bufs=3))
    sqpool = ctx.enter_context(tc.tile_pool(name="sq", bufs=2))
    invchpool = ctx.enter_context(tc.tile_pool(name="invch", bufs=2))
    bandpool = ctx.enter_context(tc.tile_pool(name="band", bufs=2))
    psmm = ctx.enter_context(tc.tile_pool(name="psmm", bufs=3, space="PSUM"))
    psaux = ctx.enter_context(tc.tile_pool(name="psaux", bufs=2, space="PSUM"))
    dramp = ctx.enter_context(tc.tile_pool(name="dramp", bufs=1, space="DRAM"))

    # ---- persistent small tiles ----
    lab_bc = singles.tile([128, NT * 128 + BAND - 128], F32)   # [128, 1280]
    lab_rows = singles.tile([128, NT], F32)
    eye = singles.tile([128, BAND], F32)
    ones_col = singles.tile([128, 1], F32R)
    ones_row = singles.tile([1, 128], F32R)
    asum = singles.tile([128, NT * (B // (2 * CH))], F32)   # per-chunk exp sums
    nposS = singles.tile([128, NT], F32)
    psumS = singles.tile([128, NT], F32)
    eselfS = singles.tile([128, NT], F32)
    pmES = singles.tile([128, NT], F32)
    top8s = singles.tile([128, NT * 8], F32)
    outsb = singles.tile([128, 32], F32)

    ones_col_f = singles.tile([128, 1], F32)
    ones_row_f = singles.tile([1, 128], F32)
    nc.gpsimd.memset(ones_col_f[:], 1.0)
    nc.gpsimd.memset(ones_row_f[:], 1.0)
    nc.vector.tensor_copy(out=ones_col[:], in_=ones_col_f[:])
    nc.vector.tensor_copy(out=ones_row[:], in_=ones_row_f[:])

    # ---- input DMAs ----
    # et_raw shares the "big" pool with the later E tiles: the raw
    # operands die once normalized, freeing both slots for E.
    et_raw = [
        bigpool.tile([128, B], F32, tag="big", name=f"etraw{_k}")
        for _k in range(KT)
    ]
    for ch in range(NCH):
        for k in range(KT):
            nc.sync.dma_start(
                et_raw[k][:, ch * CH:(ch + 1) * CH],
                et_d[k * 128:(k + 1) * 128, ch * CH:(ch + 1) * CH],
            )
    nc.sync.dma_start(eye[:], eye_d[:, :])
    nc.sync.dma_start(
        lab_rows[:],
        lab_d[0:1, 128:128 + RPC].rearrange("o (t p) -> o p t", p=128),
    )
    labrow1 = singles.tile([1, lab_bc.shape[1]], F32R)
    nc.sync.dma_start(labrow1[:], lab_d[0:1, 0:lab_bc.shape[1]].bitcast(F32R))
    # broadcast labels across partitions via ones-matmul (fp32: exact)
    for lch in range(0, lab_bc.shape[1], CH):
        w = min(CH, lab_bc.shape[1] - lch)
        ps = psaux.tile([128, CH], F32, tag="aux", name=f"labps{lch}")
        nc.tensor.matmul(
            ps[:, 0:w], ones_row[:], labrow1[0:1, lch:lch + w],
            start=True, stop=True,
        )
        nc.scalar.activation(lab_bc[:, lch:lch + w], ps[:, 0:w], AF.Copy)

    # ---- column norms: n2 -> [128,64] -> inv -> broadcast -> normalize ----
    n2_dram = dramp.tile([1, B], F32)
    inv_dram = dramp.tile([1, B], F32)
    n2pt = singles.tile([128, B // 128], F32)
    invpt = singles.tile([128, B // 128], F32)
    etn = [etnpool.tile([128, B], F32R, tag="etn", name=f"etn{_k}") for _k in range(KT)]
    for ch in range(NCH):
        ps = psaux.tile([1, CH], F32, tag="aux", name=f"n2ps{ch}")
        for k in range(KT):
            sq = sqpool.tile([128, CH], F32R, tag="sq")
            nc.vector.scalar_tensor_tensor(
                out=sq[:],
                in0=et_raw[k][:, ch * CH:(ch + 1) * CH],
                scalar=1.0,
                in1=et_raw[k][:, ch * CH:(ch + 1) * CH],
                op0=ALU.mult, op1=ALU.mult,
            )
            nc.tensor.matmul(
                ps[:], ones_col[:], sq[:],
                start=(k == 0), stop=(k == KT - 1),
            )
        n2ch = invchpool.tile([1, CH], F32, tag="n2ch", name=f"n2ch{ch}")
        nc.scalar.activation(n2ch[0:1, :], ps[0:1, :], AF.Copy)
        nc.sync.dma_start(n2_dram[0:1, ch * CH:(ch + 1) * CH], n2ch[:])

    # inv roundtrip in 4 pipelined blocks: each gather only waits on its
    # quarter of the n2 chunks, and downstream invch loads can start as
    # soon as their block's scatter lands. (p t) orientation keeps every
    # partition's transfer contiguous.
    NB = 4
    BW = B // NB
    for blk in range(NB):
        c0, c1 = blk * (64 // NB), (blk + 1) * (64 // NB)
        nc.sync.dma_start(
            n2pt[:, c0:c1],
            n2_dram[0, blk * BW:(blk + 1) * BW].rearrange(
                "(p t) -> p t", p=128
            ),
        )
        nc.scalar.activation(n2pt[:, c0:c1], n2pt[:, c0:c1], AF.Sqrt)
        nc.vector.tensor_scalar_max(n2pt[:, c0:c1], n2pt[:, c0:c1], 1e-12)
        nc.vector.reciprocal(invpt[:, c0:c1], n2pt[:, c0:c1])
        nc.sync.dma_start(
            inv_dram[0, blk * BW:(blk + 1) * BW].rearrange(
                "(p t) -> p t", p=128
            ),
            invpt[:, c0:c1],
        )

    # broadcast inv across partitions (PE), evac on ACT, scale on DVE at 2x
    for ch in range(NCH):
        invch = invchpool.tile([1, CH], F32R, tag="invch")
        nc.sync.dma_start(
            invch[:], inv_dram[0:1, ch * CH:(ch + 1) * CH].bitcast(F32R)
        )
        ps2 = psaux.tile([128, CH], F32, tag="aux", name=f"bcps{ch}")
        nc.tensor.matmul(
            ps2[:], ones_row[:], invch[0:1, :],
            start=True, stop=True,
        )
        invb = sqpool.tile([128, CH], F32, tag="invb", name=f"invb{ch}")
        nc.scalar.activation(invb[:], ps2[:], AF.Copy)
        for k in range(KT):
            nc.vector.scalar_tensor_tensor(
                out=etn[k][:, ch * CH:(ch + 1) * CH],
                in0=invb[:], scalar=1.0,
                in1=et_raw[k][:, ch * CH:(ch + 1) * CH],
                op0=ALU.mult, op1=ALU.mult,
            )

    # ---- main loop over row tiles ----
    W = 2 * CH   # 1024-wide PSUM chunks (2 banks) amortize ACT init cost
    NW = B // W
    for t in range(NT):
        E = bigpool.tile([128, B], F32, tag="big")
        lo = 128 + t * 128
        for w in range(NW):
            ps = psmm.tile([128, W], F32, tag="mm")
            for half in range(2):
                c0 = w * W + half * CH
                for k in range(KT):
                    nc.tensor.matmul(
                        ps[:, half * CH:(half + 1) * CH],
                        etn[k][:, lo:lo + 128],
                        etn[k][:, c0:c0 + CH],
                        start=(k == 0), stop=(k == KT - 1),
                    )
            nc.scalar.activation(
                E[:, w * W:(w + 1) * W], ps[:], AF.Exp,
                scale=INVT,
                accum_out=asum[:, t * NW + w:t * NW + w + 1],
            )

        # band [t*128, t*128+384) holds all same-class cols of these rows
        bl = t * 128
        Eb = E[:, bl:bl + BAND]
        mask = bandpool.tile([128, BAND], F32, tag="mask")
        maskx = bandpool.tile([128, BAND], F32, tag="maskx")
        epos = bandpool.tile([128, BAND], F32, tag="epos")
        scr = bandpool.tile([128, BAND], F32, tag="scr")
        nc.vector.tensor_scalar(
            out=mask[:], in0=lab_bc[:, bl:bl + BAND],
            scalar1=lab_rows[:, t:t + 1], scalar2=None, op0=ALU.is_equal,
        )
        # maskx = mask - eye ; npos = rowsum(maskx)
        nc.vector.scalar_tensor_tensor(
            out=maskx[:], in0=eye[:], scalar=-1.0, in1=mask[:],
            op0=ALU.mult, op1=ALU.add,
            accum_out=nposS[:, t:t + 1],
        )
        # epos = maskx * E ; pos_sum = rowsum(epos)
        nc.vector.scalar_tensor_tensor(
            out=epos[:], in0=maskx[:], scalar=1.0, in1=Eb,
            op0=ALU.mult, op1=ALU.mult,
            accum_out=psumS[:, t:t + 1],
        )
        # e_self = rowsum(eye * E)
        nc.vector.scalar_tensor_tensor(
            out=scr[:], in0=eye[:], scalar=1.0, in1=Eb,
            op0=ALU.mult, op1=ALU.mult,
            accum_out=eselfS[:, t:t + 1],
        )
        # pos_max in E-space
        nc.vector.tensor_reduce(
            out=pmES[:, t:t + 1], in_=epos[:], axis=AX.X, op=ALU.max
        )
        # mask same-class (incl self) out of E for the negatives top-k
        nc.vector.scalar_tensor_tensor(
            out=Eb, in0=mask[:], scalar=NEG_BIG, in1=Eb,
            op0=ALU.mult, op1=ALU.add,
        )
        # top-8 negatives (descending, with duplicates) over the full row
        nc.vector.max(top8s[:, t * 8:(t + 1) * 8], E[:, :])

    # ---- epilogue: per-row losses on [128, NT] tiles ----
    ep = ctx.enter_context(tc.tile_pool(name="ep", bufs=1))
    allsum = ep.tile([128, NT], F32)
    rp = ep.tile([128, NT], F32)
    ratio = ep.tile([128, NT], F32)
    Lb = ep.tile([128, NT], F32)
    hp = ep.tile([128, NT], F32)
    pmx = ep.tile([128, NT], F32)
    l3 = ep.tile([128, NT * 3], F32)
    s123 = ep.tile([128, NT], F32)
    u = ep.tile([128, NT], F32)
    v = ep.tile([128, NT], F32)

    nc.vector.tensor_reduce(
        out=allsum[:], in_=asum[:].rearrange("p (t n) -> p t n", n=B // (2 * CH)),
        axis=AX.X, op=ALU.add,
    )
    # allsum excludes self; +1e-10 for the reference's denominator eps
    nc.vector.tensor_tensor(
        out=allsum[:], in0=allsum[:], in1=eselfS[:], op=ALU.subtract
    )
    nc.vector.tensor_scalar_add(allsum[:], allsum[:], 1e-10)
    nc.vector.reciprocal(rp[:], allsum[:])
    nc.vector.scalar_tensor_tensor(
        out=ratio[:], in0=psumS[:], scalar=1.0, in1=rp[:],
        op0=ALU.mult, op1=ALU.mult,
    )
    nc.vector.tensor_scalar_add(ratio[:], ratio[:], 1e-10)
    nc.scalar.activation(Lb[:], ratio[:], AF.Ln)
    # hp = npos > 0
    nc.vector.tensor_scalar(
        out=hp[:], in0=nposS[:], scalar1=0.5, scalar2=None, op0=ALU.is_ge
    )
    # pos_max (ln units); rows with no positives get a junk finite value
    nc.vector.tensor_scalar_max(pmES[:], pmES[:], 1e-30)
    nc.scalar.activation(pmx[:], pmES[:], AF.Ln)
    # top-3 negative sims (ln units)
    nc.scalar.activation(
        l3[:].rearrange("p (t k) -> p t k", k=3),
        top8s[:].rearrange("p (t k) -> p t k", k=8)[:, :, 0:3],
        AF.Ln,
    )
    nc.vector.tensor_reduce(
        out=s123[:], in_=l3[:].rearrange("p (t k) -> p t k", k=3),
        axis=AX.X, op=ALU.add,
    )
    # ln(E) is already in the reference's T-scaled sim domain.
    # hard: h = relu(s123/3 - pmx + MARGIN) * hp
    nc.vector.scalar_tensor_tensor(
        out=u[:], in0=s123[:], scalar=1.0 / 3.0, in1=pmx[:],
        op0=ALU.mult, op1=ALU.subtract,
    )
    nc.vector.tensor_scalar(
        out=v[:], in0=u[:], scalar1=MARGIN, scalar2=0.0,
        op0=ALU.add, op1=ALU.max,
    )
    nc.vector.tensor_tensor(
        out=outsb[:, 16:24], in0=v[:], in1=hp[:], op=ALU.mult
    )
    # margin: m = relu(s1 - pmx + MARGIN) * hp
    nc.vector.scalar_tensor_tensor(
        out=u[:], in0=l3[:].rearrange("p (t k) -> p t k", k=3)[:, :, 0],
        scalar=1.0, in1=pmx[:], op0=ALU.mult, op1=ALU.subtract,
    )
    nc.vector.tensor_scalar(
        out=v[:], in0=u[:], scalar1=MARGIN, scalar2=0.0,
        op0=ALU.add, op1=ALU.max,
    )
    nc.vector.tensor_tensor(
        out=outsb[:, 24:32], in0=v[:], in1=hp[:], op=ALU.mult
    )
    # basic: -ln(ratio) * hp
    nc.vector.scalar_tensor_tensor(
        out=outsb[:, 0:8], in0=Lb[:], scalar=-1.0, in1=hp[:],
        op0=ALU.mult, op1=ALU.mult,
    )
    nc.vector.tensor_copy(out=outsb[:, 8:16], in_=hp[:])

    nc.sync.dma_start(out_d[:, :], outsb[:])


def _prep_inputs(embeddings, labels):
    e = np.ascontiguousarray(np.asarray(embeddings), dtype=np.float32)
    lab = np.asarray(labels)
    assert e.shape == (B, D) and lab.shape == (B,)
    perm = np.argsort(lab, kind="stable")
    e_s = e[perm]
    lab_s = lab[perm].astype(np.float32)
    counts = np.bincount(lab[perm].astype(np.int64))
    assert counts.max() <= 128, f"class size {counts.max()} > band margin"

    eye = np.zeros((128, BAND), dtype=np.float32)
    eye[np.arange(128), 128 + np.arange(128)] = 1.0

    in_maps = []
    for c in range(NC):
        s = (c * RPC - 128) % B
        er = np.concatenate([e_s[s:], e_s[:s]], axis=0)
        lr = np.concatenate([lab_s[s:], lab_s[:s]])
        in_maps.append(
            {
                "et": np.ascontiguousarray(er.T),
                "labf": np.ascontiguousarray(lr[None, :]),
                "eye": eye,
            }
        )
    return in_maps


def _combine(results):
    SA = np.float32(0.0)
    SB = np.float32(0.0)
    SC = np.float32(0.0)
    SD = np.float32(0.0)
    for r in results:
        o = r["out"].astype(np.float32)
        SA += o[:, 0:8].sum(dtype=np.float32)
        SB += o[:, 8:16].sum(dtype=np.float32)
        SC += o[:, 16:24].sum(dtype=np.float32)
        SD += o[:, 24:32].sum(dtype=np.float32)
    nhp = max(SB, np.float32(1.0))
    basic = SA / nhp
    hard = SC / nhp
    margin = SD / nhp if SB > 0 else np.float32(0.0)
    total = basic + np.float32(0.5) * hard + np.float32(0.1) * margin
    return np.asarray(total, dtype=np.float32)


def kernel(embeddings, labels):
    in_maps = _prep_inputs(embeddings, labels)
    nc = _build_program()
    res = run_bass_kernel_spmd(nc, in_maps, core_ids=list(range(NC)))
    return _combine(res.results)



# revision 46
# speedup vs baseline: 1.6576x; 1.6576x over previous
"""EnhancedContrastiveLoss on 8 Trainium2 NeuronCores (Bass/Tile).

Asymmetric normalization + bf16 datapath, tuned so the scalar engine's
exp stream is the only saturated resource in steady state.

Host side (layout only): sort samples by label; shard 1024 rows/core with
a per-core column rotation so every core sees its rows' class neighborhood
at the same local columns (SPMD-constant addressing); ship et=[D,B] bf16,
labels fp16, eye bf16.

Device side (per core):
  * column norms: sq = et*et (half on ACT Square, half on DVE bf16 tt);
    n2 = per-128-column-group single-column matmuls with sq STATIONARY,
    landing n2 as [128,64] in PSUM directly; inv = rsqrt(max(n2,1e-24))
    via DVE-only bit-trick + 2 Newton steps (no ACT Sqrt table load);
    DMA-rearranged to a [1,B] row and partition-broadcast on GPSIMD.
  * only the RHS matmul operand is normalized (etn = et*inv); rows stay
    raw and exp applies a per-row scale inv_i/T on ACT:
    exp(raw_ij * inv_i / T) == exp(sim_ij / T) exactly.
  * sim row-tiles as 4x [128,2048] PSUM chunks (bf16 matmuls, 512-wide);
    ACT exp with fused row-sum accum. Chunk 0 (which contains the class
    band) is written fp32 so the accumulated row sums match the stored
    values bit-exactly; chunks 1-3 are bf16 for the 2x DVE max tree.
  * band stats in one pass: scrm = mask*E0f; its InstMax top-8 yields
    eself (top-1: sim_ii==1 dominates the band) and pos_max (top-2);
    its fused accum yields pos_sum + eself.
  * negatives top-8: disjoint-window bf16 tensor_tensor max tree (DVE 2x)
    folded to 512 candidates, then InstMax; the chunk-0/1 side folds
    early so only the chunk-2/3 side runs after the tile's last exp.
Host side: combine 8 cores' [128,32] partials into the 3 scalar losses.

Note: InstTensorTensorReduce aborts on this runtime (even all-fp32), and
generic tensor ops (stt/copy/reduce) fail the Pool-engine ISA check --
band reductions use scalar_tensor_tensor accum_out on DVE instead, and
GPSIMD only runs memset/partition_broadcast.
"""

import numpy as np
import ml_dtypes
from contextlib import ExitStack

import concourse.bass as bass
import concourse.mybir as mybir
from concourse import bacc, tile
from concourse.bass_utils import run_bass_kernel_spmd

F32 = mybir.dt.float32
BF16 = mybir.dt.bfloat16
F16 = mybir.dt.float16
I32 = mybir.dt.int32
AF = mybir.ActivationFunctionType
ALU = mybir.AluOpType
AX = mybir.AxisListType

B = 8192
D = 256
NC = 8
RPC = B // NC          # rows per core (1024)
NT = RPC // 128        # row tiles per core (8)
KT = D // 128          # K tiles (2)
BAND = 384
LABW = NT * 128 + BAND - 128   # 1280: label window needed on device
W = 2048               # PSUM chunk width (4 banks)
NW = B // W            # 4
MM = 512               # matmul moving-dim max
TEMP = 0.07
MARGIN = 0.2
INVT = 1.0 / TEMP
NEG_BIG = -1.0e30

_CACHE = {}


def _build_program():
    if "nc" in _CACHE:
        return _CACHE["nc"]
    nc = bacc.Bacc(
        "TRN2", target_bir_lowering=False, debug=False, num_devices=NC
    )
    et_d = nc.dram_tensor("et", [D, B], BF16, kind="ExternalInput").ap()
    lab_d = nc.dram_tensor("labf", [1, LABW], F16, kind="ExternalInput").ap()
    eye_d = nc.dram_tensor("eye", [128, BAND], BF16, kind="ExternalInput").ap()
    out_d = nc.dram_tensor("out", [128, 32], F32, kind="ExternalOutput").ap()

    with tile.TileContext(nc) as tc:
        with ExitStack() as ctx:
            _body(ctx, tc, et_d, lab_d, eye_d, out_d)

    nc.finalize()
    _CACHE["nc"] = nc
    return nc


def _body(ctx, tc, et_d, lab_d, eye_d, out_d):
    nc = tc.nc

    singles = ctx.enter_context(tc.tile_pool(name="singles", bufs=1))
    etpool = ctx.enter_context(tc.tile_pool(name="et", bufs=1))
    epool = ctx.enter_context(tc.tile_pool(name="E", bufs=3))
    sqpool = ctx.enter_context(tc.tile_pool(name="sq", bufs=2))
    invbpool = ctx.enter_context(tc.tile_pool(name="invb", bufs=2))
    bandpool = ctx.enter_context(tc.tile_pool(name="band", bufs=2))
    treepool = ctx.enter_context(tc.tile_pool(name="tree", bufs=2))
    psmm = ctx.enter_context(tc.tile_pool(name="psmm", bufs=2, space="PSUM"))
    dramp = ctx.enter_context(tc.tile_pool(name="dramp", bufs=1, space="DRAM"))

    # ---- persistent tiles ----
    ones_col = singles.tile([128, 1], BF16)
    lab_bc = singles.tile([128, LABW], F16)
    lab_rows16 = singles.tile([128, NT], F16)
    lab_rows = singles.tile([128, NT], F32)
    eye = singles.tile([128, BAND], BF16)
    n2pt = singles.tile([128, B // 128], F32)     # [128, 64]
    invpt = singles.tile([128, B // 128], F32)
    invptb = singles.tile([128, B // 128], BF16)
    ish = singles.tile([128, B // 128], I32)
    magic = singles.tile([128, B // 128], I32)
    one_i = singles.tile([128, B // 128], I32)
    nt1 = singles.tile([128, B // 128], F32)
    nt2 = singles.tile([128, B // 128], F32)
    invrowb = singles.tile([1, B], BF16)          # bcast source, partition 0
    invrows8 = singles.tile([128, NT], BF16)
    invrowsT = singles.tile([128, NT], F32)       # inv_i / T per row tile
    asum = singles.tile([128, NT * NW], F32)
    smS = singles.tile([128, NT], F32)      # rowsum of mask (npos + 1)
    msumS = singles.tile([128, NT], F32)    # rowsum of mask*E (pos_sum + eself)
    top8b = singles.tile([128, NT * 8], F32)  # band top-8: [0]=eself, [1]=pos_max
    top8s = singles.tile([128, NT * 8], BF16)
    outsb = singles.tile([128, 32], F32)

    nc.gpsimd.memset(ones_col[:], 1.0)
    nc.gpsimd.memset(magic[:], 0x5F3759DF)
    nc.gpsimd.memset(one_i[:], 1)

    # ---- input DMAs ----
    et = [etpool.tile([128, B], BF16, name=f"et{k}") for k in range(KT)]
    etn = [etpool.tile([128, B], BF16, name=f"etn{k}") for k in range(KT)]
    labrow = singles.tile([1, LABW], F16)
    for c in range(NW):
        for k in range(KT):
            nc.sync.dma_start(
                et[k][:, c * W:(c + 1) * W],
                et_d[k * 128:(k + 1) * 128, c * W:(c + 1) * W],
            )

    # ---- column norms + normalize, chunk-pipelined ----
    # sq = et*et (bf16, DVE 2x); n2[m] = sum_k sq[k,m] via single-column
    # matmuls with sq as the STATIONARY operand -> n2 lands as [128, 64]
    # in PSUM directly (n2[128g+p] at [p, g]); inv computed per chunk on
    # the fat layout, DMA-rearranged to a [1,B] row for the GPSIMD
    # broadcast, then etn = et * inv (DVE tt, bf16 2x).
    G = W // 128  # column groups per chunk (16)
    # shares the "mm" slot rotation (slot 0); freed before sim mm t0w1 needs it
    n2ps = psmm.tile([128, B // 128], F32, tag="mm", name="n2ps")
    inv_dram = dramp.tile([1, B], BF16)
    for c in range(NW):
        cs = slice(c * G, (c + 1) * G)
        sqs = []
        for k in range(KT):
            sq = sqpool.tile([128, W], BF16, tag=f"sq{k}", name=f"sq{k}_{c}")
            if k == 0:
                # ACT is idle during the preamble; all Squares precede the
                # first Exp so the table loads only once.
                nc.scalar.activation(
                    sq[:], et[k][:, c * W:(c + 1) * W], AF.Square
                )
            else:
                nc.vector.tensor_tensor(
                    out=sq[:],
                    in0=et[k][:, c * W:(c + 1) * W],
                    in1=et[k][:, c * W:(c + 1) * W],
                    op=ALU.mult,
                )
            sqs.append(sq)
        for gl in range(G):
            g = c * G + gl
            for k in range(KT):
                nc.tensor.matmul(
                    n2ps[:, g:g + 1],
                    sqs[k][:, gl * 128:(gl + 1) * 128],
                    ones_col[:],
                    start=(k == 0), stop=(k == KT - 1),
                )
        # inv = rsqrt(max(n2, 1e-24)) on this chunk's [128, 16] slice --
        # DVE-only (bit-trick seed + 2 Newton steps) so ACT never loads a
        # Sqrt table between the main-loop Exp activations.
        nc.vector.tensor_scalar(
            out=n2pt[:, cs], in0=n2ps[:, cs], scalar1=1e-24, scalar2=None,
            op0=ALU.max,
        )
        nc.vector.tensor_tensor(
            out=ish[:, cs], in0=n2pt[:, cs].bitcast(I32), in1=one_i[:, cs],
            op=ALU.logical_shift_right,
        )
        nc.vector.tensor_tensor(
            out=invpt[:, cs].bitcast(I32), in0=magic[:, cs], in1=ish[:, cs],
            op=ALU.subtract,
        )
        for _ in range(2):
            nc.vector.tensor_tensor(
                out=nt1[:, cs], in0=invpt[:, cs], in1=invpt[:, cs],
                op=ALU.mult,
            )
            nc.vector.scalar_tensor_tensor(
                out=nt2[:, cs], in0=n2pt[:, cs], scalar=-0.5, in1=nt1[:, cs],
                op0=ALU.mult, op1=ALU.mult,
            )
            nc.vector.tensor_scalar_add(nt2[:, cs], nt2[:, cs], 1.5)
            nc.vector.tensor_tensor(
                out=invpt[:, cs], in0=invpt[:, cs], in1=nt2[:, cs],
                op=ALU.mult,
            )
        nc.vector.tensor_copy(out=invptb[:, cs], in_=invpt[:, cs])
        nc.sync.dma_start(
            inv_dram[0, c * W:(c + 1) * W].rearrange("(t p) -> p t", p=128),
            invptb[:, cs],
        )
        nc.sync.dma_start(
            invrowb[0:1, c * W:(c + 1) * W],
            inv_dram[0:1, c * W:(c + 1) * W],
        )
        if c == 0:
            # per-row scale inv_i / T (rows 128..1152 live in chunk 0)
            nc.sync.dma_start(
                invrows8[:],
                inv_dram[0:1, 128:128 + RPC].rearrange(
                    "o (t p) -> o p t", p=128
                ),
            )
            nc.vector.tensor_scalar_mul(invrowsT[:], invrows8[:], INVT)
        invb = invbpool.tile([128, W], BF16, tag="invb", name=f"invb{c}")
        nc.gpsimd.partition_broadcast(invb[:], invrowb[0:1, c * W:(c + 1) * W])
        for k in range(KT):
            nc.vector.tensor_tensor(
                out=etn[k][:, c * W:(c + 1) * W],
                in0=et[k][:, c * W:(c + 1) * W],
                in1=invb[:],
                op=ALU.mult,
            )
        if c == 0:
            nc.sync.dma_start(eye[:], eye_d[:, :])
            nc.sync.dma_start(labrow[:], lab_d[0:1, :])
            nc.sync.dma_start(
                lab_rows16[:],
                lab_d[0:1, 128:128 + RPC].rearrange("o (t p) -> o p t", p=128),
            )
            nc.vector.tensor_copy(out=lab_rows[:], in_=lab_rows16[:])
            nc.gpsimd.partition_broadcast(lab_bc[:], labrow[0:1, :])

    # ---- main loop over row tiles ----
    for t in range(NT):
        # E chunk 0 is written fp32 (the label band lives there): the
        # accumulated row sum then matches the stored values bit-exactly,
        # so eself/pos_sum subtract cleanly without any re-exp. Chunks
        # 1-3 stay bf16 for the 2x DVE max tree.
        E0f = epool.tile([128, W], F32, tag="E0f")
        E = epool.tile([128, B - W], BF16, tag="E")
        lo = 128 + t * 128
        bl = t * 128
        for w in range(NW):
            ps = psmm.tile([128, W], F32, tag="mm", name=f"mm{t}_{w}")
            for j in range(W // MM):
                c0 = w * W + j * MM
                for k in range(KT):
                    nc.tensor.matmul(
                        ps[:, j * MM:(j + 1) * MM],
                        et[k][:, lo:lo + 128],
                        etn[k][:, c0:c0 + MM],
                        start=(k == 0), stop=(k == KT - 1),
                    )
            dst = E0f[:] if w == 0 else E[:, (w - 1) * W:w * W]
            nc.scalar.activation(
                dst, ps[:], AF.Exp,
                scale=invrowsT[:, t:t + 1],
                accum_out=asum[:, t * NW + w:t * NW + w + 1],
            )

        # band stats: positives live in cols [bl, bl+BAND)
        mask = bandpool.tile([128, BAND], BF16, tag="mask")
        scrm = bandpool.tile([128, BAND], F32, tag="scrm")
        nc.vector.tensor_scalar(
            out=mask[:], in0=lab_bc[:, bl:bl + BAND],
            scalar1=lab_rows[:, t:t + 1], scalar2=0.0, op0=ALU.is_equal,
            op1=ALU.add, accum_out=smS[:, t:t + 1],
        )
        # scrm = mask * E32: top-1 = eself (sim_ii==1 dominates the band),
        # top-2 = pos_max; rowsum = pos_sum + eself
        # (stt, not tensor_tensor_reduce: InstTensorTensorReduce aborts on
        # this runtime even in all-fp32 form)
        nc.vector.scalar_tensor_tensor(
            out=scrm[:], in0=mask[:], scalar=1.0, in1=E0f[:, bl:bl + BAND],
            op0=ALU.mult, op1=ALU.mult,
            accum_out=msumS[:, t:t + 1],
        )
        nc.vector.max(top8b[:, t * 8:(t + 1) * 8], scrm[:])
        # mask same-class (incl self) out of E for the negatives top-k
        nc.vector.scalar_tensor_tensor(
            out=E0f[:, bl:bl + BAND], in0=mask[:], scalar=NEG_BIG,
            in1=E0f[:, bl:bl + BAND], op0=ALU.mult, op1=ALU.add,
        )

        # negatives top-8 over disjoint window maxes; the chunk-0/1 side
        # folds to 512 early so only the chunk-2/3 side + merge runs after
        # the last exp of the tile.
        m01 = treepool.tile([128, W], BF16, tag="m01")
        m23 = treepool.tile([128, W], BF16, tag="m23")
        m2 = treepool.tile([128, W // 2], BF16, tag="m2")
        m3 = treepool.tile([128, W // 4], BF16, tag="m3")
        m2b = treepool.tile([128, W // 2], BF16, tag="m2b")
        m3b = treepool.tile([128, W // 4], BF16, tag="m3b")
        nc.vector.tensor_tensor(
            out=m01[:], in0=E0f[:], in1=E[:, 0:W], op=ALU.max
        )
        nc.vector.tensor_tensor(
            out=m2[:], in0=m01[:, 0:W // 2], in1=m01[:, W // 2:W], op=ALU.max
        )
        nc.vector.tensor_tensor(
            out=m3[:], in0=m2[:, 0:W // 4], in1=m2[:, W // 4:W // 2],
            op=ALU.max,
        )
        nc.vector.tensor_tensor(
            out=m23[:], in0=E[:, W:2 * W], in1=E[:, 2 * W:3 * W], op=ALU.max
        )
        nc.vector.tensor_tensor(
            out=m2b[:], in0=m23[:, 0:W // 2], in1=m23[:, W // 2:W],
            op=ALU.max,
        )
        nc.vector.tensor_tensor(
            out=m3b[:], in0=m2b[:, 0:W // 4], in1=m2b[:, W // 4:W // 2],
            op=ALU.max,
        )
        nc.vector.tensor_tensor(
            out=m3[:], in0=m3[:], in1=m3b[:], op=ALU.max
        )
        nc.vector.max(top8s[:, t * 8:(t + 1) * 8], m3[:])


    # ---- epilogue: per-row losses on [128, NT] tiles ----
    ep = ctx.enter_context(tc.tile_pool(name="ep", bufs=1))
    allsum = ep.tile([128, NT], F32)
    eself = ep.tile([128, NT], F32)
    psumS = ep.tile([128, NT], F32)
    pmES = ep.tile([128, NT], F32)
    rp = ep.tile([128, NT], F32)
    ratio = ep.tile([128, NT], F32)
    Lb = ep.tile([128, NT], F32)
    hp = ep.tile([128, NT], F32)
    pmx = ep.tile([128, NT], F32)
    l3 = ep.tile([128, NT * 3], F32)
    s123 = ep.tile([128, NT], F32)
    u = ep.tile([128, NT], F32)
    v = ep.tile([128, NT], F32)

    t8v = top8b[:].rearrange("p (t k) -> p t k", k=8)
    nc.vector.tensor_copy(out=eself[:], in_=t8v[:, :, 0])
    nc.vector.tensor_copy(out=pmES[:], in_=t8v[:, :, 1])
    nc.vector.tensor_reduce(
        out=allsum[:], in_=asum[:].rearrange("p (t n) -> p t n", n=NW),
        axis=AX.X, op=ALU.add,
    )
    # allsum excludes self; +1e-10 for the reference's denominator eps
    nc.vector.tensor_tensor(
        out=allsum[:], in0=allsum[:], in1=eself[:], op=ALU.subtract
    )
    nc.vector.tensor_scalar_add(allsum[:], allsum[:], 1e-10)
    nc.vector.reciprocal(rp[:], allsum[:])
    # pos_sum = msum - eself
    nc.vector.tensor_tensor(
        out=psumS[:], in0=msumS[:], in1=eself[:], op=ALU.subtract
    )
    nc.vector.scalar_tensor_tensor(
        out=ratio[:], in0=psumS[:], scalar=1.0, in1=rp[:],
        op0=ALU.mult, op1=ALU.mult,
    )
    nc.vector.tensor_scalar_add(ratio[:], ratio[:], 1e-10)
    nc.scalar.activation(Lb[:], ratio[:], AF.Ln)
    # hp = npos > 0  <=>  summask >= 2 (summask = npos + 1, integer-valued)
    nc.vector.tensor_scalar(
        out=hp[:], in0=smS[:], scalar1=1.5, scalar2=None, op0=ALU.is_ge
    )
    # pos_max (ln units); rows with no positives get a junk finite value
    nc.vector.tensor_scalar_max(pmES[:], pmES[:], 1e-30)
    nc.scalar.activation(pmx[:], pmES[:], AF.Ln)
    # top-3 negative sims (ln units)
    nc.scalar.activation(
        l3[:].rearrange("p (t k) -> p t k", k=3),
        top8s[:].rearrange("p (t k) -> p t k", k=8)[:, :, 0:3],
        AF.Ln,
    )
    nc.vector.tensor_reduce(
        out=s123[:], in_=l3[:].rearrange("p (t k) -> p t k", k=3),
        axis=AX.X, op=ALU.add,
    )
    # hard: h = relu(s123/3 - pmx + MARGIN) * hp
    nc.vector.scalar_tensor_tensor(
        out=u[:], in0=s123[:], scalar=1.0 / 3.0, in1=pmx[:],
        op0=ALU.mult, op1=ALU.subtract,
    )
    nc.vector.tensor_scalar(
        out=v[:], in0=u[:], scalar1=MARGIN, scalar2=0.0,
        op0=ALU.add, op1=ALU.max,
    )
    nc.vector.tensor_tensor(
        out=outsb[:, 16:24], in0=v[:], in1=hp[:], op=ALU.mult
    )
    # margin: m = relu(s1 - pmx + MARGIN) * hp
    nc.vector.scalar_tensor_tensor(
        out=u[:], in0=l3[:].rearrange("p (t k) -> p t k", k=3)[:, :, 0],
        scalar=1.0, in1=pmx[:], op0=ALU.mult, op1=ALU.subtract,
    )
    nc.vector.tensor_scalar(
        out=v[:], in0=u[:], scalar1=MARGIN, scalar2=0.0,
        op0=ALU.add, op1=ALU.max,
    )
    nc.vector.tensor_tensor(
        out=outsb[:, 24:32], in0=v[:], in1=hp[:], op=ALU.mult
    )
    # basic: -ln(ratio) * hp
    nc.vector.scalar_tensor_tensor(
        out=outsb[:, 0:8], in0=Lb[:], scalar=-1.0, in1=hp[:],
        op0=ALU.mult, op1=ALU.mult,
    )
    nc.vector.tensor_copy(out=outsb[:, 8:16], in_=hp[:])

    nc.sync.dma_start(out_d[:, :], outsb[:])


def _prep_inputs(embeddings, labels):
    e = np.ascontiguousarray(np.asarray(embeddings), dtype=np.float32)
    lab = np.asarray(labels)
    assert e.shape == (B, D) and lab.shape == (B,)
    perm = np.argsort(lab, kind="stable")
    e_s = e[perm]
    lab_s = lab[perm].astype(np.float16)
    counts = np.bincount(lab[perm].astype(np.int64))
    assert counts.max() <= 128, f"class size {counts.max()} > band margin"

    eye = np.zeros((128, BAND), dtype=ml_dtypes.bfloat16)
    eye[np.arange(128), 128 + np.arange(128)] = 1.0

    in_maps = []
    for c in range(NC):
        s = (c * RPC - 128) % B
        er = np.concatenate([e_s[s:], e_s[:s]], axis=0)
        lr = np.concatenate([lab_s[s:], lab_s[:s]])
        in_maps.append(
            {
                "et": np.ascontiguousarray(er.T).astype(ml_dtypes.bfloat16),
                "labf": np.ascontiguousarray(lr[None, :LABW]),
                "eye": eye,
            }
        )
    return in_maps


def _combine(results):
    SA = np.float32(0.0)
    SB = np.float32(0.0)
    SC = np.float32(0.0)
    SD = np.float32(0.0)
    for r in results:
        o = r["out"].astype(np.float32)
        SA += o[:, 0:8].sum(dtype=np.float32)
        SB += o[:, 8:16].sum(dtype=np.float32)
        SC += o[:, 16:24].sum(dtype=np.float32)
        SD += o[:, 24:32].sum(dtype=np.float32)
    nhp = max(SB, np.float32(1.0))
    basic = SA / nhp
    hard = SC / nhp
    margin = SD / nhp if SB > 0 else np.float32(0.0)
    total = basic + np.float32(0.5) * hard + np.float32(0.1) * margin
    return np.asarray(total, dtype=np.float32)


def kernel(embeddings, labels):
    in_maps = _prep_inputs(embeddings, labels)
    nc = _build_program()
    res = run_bass_kernel_spmd(nc, in_maps, core_ids=list(range(NC)))
    return _combine(res.results)


# revision 51
# speedup vs baseline: 1.6841x; 1.0160x over previous
"""EnhancedContrastiveLoss on 8 Trainium2 NeuronCores (Bass/Tile).

Asymmetric normalization + bf16 datapath, tuned so the scalar engine's
exp stream is the only saturated resource in steady state.

Host side (layout only): sort samples by label; shard 1024 rows/core with
a per-core column rotation so every core sees its rows' class neighborhood
at the same local columns (SPMD-constant addressing); ship et=[D,B] bf16,
labels fp16, eye bf16.

Device side (per core):
  * column norms: sq = et*et (half on ACT Square, half on DVE bf16 tt);
    n2 = per-128-column-group single-column matmuls with sq STATIONARY,
    landing n2 as [128,64] in PSUM directly; inv = rsqrt(max(n2,1e-24))
    via DVE-only bit-trick + 2 Newton steps (no ACT Sqrt table load);
    DMA-rearranged to a [1,B] row and partition-broadcast on GPSIMD.
  * only the RHS matmul operand is normalized (etn = et*inv); rows stay
    raw and exp applies a per-row scale inv_i/T on ACT:
    exp(raw_ij * inv_i / T) == exp(sim_ij / T) exactly.
  * sim row-tiles as 4x [128,2048] PSUM chunks (bf16 matmuls, 512-wide);
    ACT exp with fused row-sum accum. Chunk 0 (which contains the class
    band) is written fp32 so the accumulated row sums match the stored
    values bit-exactly; chunks 1-3 are bf16 for the 2x DVE max tree.
  * band stats in one pass: scrm = mask*E0f; its InstMax top-8 yields
    eself (top-1: sim_ii==1 dominates the band) and pos_max (top-2);
    its fused accum yields pos_sum + eself.
  * negatives top-8: disjoint-window bf16 tensor_tensor max tree (DVE 2x)
    folded to 512 candidates, then InstMax; the chunk-0/1 side folds
    early so only the chunk-2/3 side runs after the tile's last exp.
Host side: combine 8 cores' [128,32] partials into the 3 scalar losses.

Note: InstTensorTensorReduce aborts on this runtime (even all-fp32), and
generic tensor ops (stt/copy/reduce) fail the Pool-engine ISA check --
band reductions use scalar_tensor_tensor accum_out on DVE instead, and
GPSIMD only runs memset/partition_broadcast.
"""

import numpy as np
import ml_dtypes
from contextlib import ExitStack

import concourse.bass as bass
import concourse.mybir as mybir
from concourse import bacc, tile
from concourse.bass_utils import run_bass_kernel_spmd

F32 = mybir.dt.float32
BF16 = mybir.dt.bfloat16
F16 = mybir.dt.float16
I32 = mybir.dt.int32
AF = mybir.ActivationFunctionType
ALU = mybir.AluOpType
AX = mybir.AxisListType

B = 8192
D = 256
NC = 8
RPC = B // NC          # rows per core (1024)
NT = RPC // 128        # row tiles per core (8)
KT = D // 128          # K tiles (2)
BAND = 384
LABW = NT * 128 + BAND - 128   # 1280: label window needed on device
W = 2048               # PSUM chunk width (4 banks)
NW = B // W            # 4
MM = 512               # matmul moving-dim max
TEMP = 0.07
MARGIN = 0.2
INVT = 1.0 / TEMP
NEG_BIG = -1.0e30

_CACHE = {}


def _build_program():
    if "nc" in _CACHE:
        return _CACHE["nc"]
    nc = bacc.Bacc(
        "TRN2", target_bir_lowering=False, debug=False, num_devices=NC
    )
    et_d = nc.dram_tensor("et", [D, B], BF16, kind="ExternalInput").ap()
    lab_d = nc.dram_tensor("labf", [1, LABW], F16, kind="ExternalInput").ap()
    eye_d = nc.dram_tensor("eye", [128, BAND], BF16, kind="ExternalInput").ap()
    out_d = nc.dram_tensor("out", [128, 32], F32, kind="ExternalOutput").ap()

    with tile.TileContext(nc) as tc:
        with ExitStack() as ctx:
            _body(ctx, tc, et_d, lab_d, eye_d, out_d)

    nc.finalize()
    _CACHE["nc"] = nc
    return nc


def _body(ctx, tc, et_d, lab_d, eye_d, out_d):
    nc = tc.nc

    singles = ctx.enter_context(tc.tile_pool(name="singles", bufs=1))
    etpool = ctx.enter_context(tc.tile_pool(name="et", bufs=1))
    epool = ctx.enter_context(tc.tile_pool(name="E", bufs=3))
    sqpool = ctx.enter_context(tc.tile_pool(name="sq", bufs=2))
    invbpool = ctx.enter_context(tc.tile_pool(name="invb", bufs=2))
    bandpool = ctx.enter_context(tc.tile_pool(name="band", bufs=2))
    treepool = ctx.enter_context(tc.tile_pool(name="tree", bufs=2))
    psmm = ctx.enter_context(tc.tile_pool(name="psmm", bufs=2, space="PSUM"))
    dramp = ctx.enter_context(tc.tile_pool(name="dramp", bufs=1, space="DRAM"))

    # ---- persistent tiles ----
    ones_col = singles.tile([128, 1], BF16)
    lab_bc = singles.tile([128, LABW], F16)
    lab_rows16 = singles.tile([128, NT], F16)
    lab_rows = singles.tile([128, NT], F32)
    eye = singles.tile([128, BAND], BF16)
    n2pt = singles.tile([128, B // 128], F32)     # [128, 64]
    invpt = singles.tile([128, B // 128], F32)
    invptb = singles.tile([128, B // 128], BF16)
    ish = singles.tile([128, B // 128], I32)
    magic = singles.tile([128, B // 128], I32)
    one_i = singles.tile([128, B // 128], I32)
    nt1 = singles.tile([128, B // 128], F32)
    nt2 = singles.tile([128, B // 128], F32)
    invrowb = singles.tile([1, B], BF16)          # bcast source, partition 0
    invrows8 = singles.tile([128, NT], BF16)
    invrowsT = singles.tile([128, NT], F32)       # inv_i / T per row tile
    asum = singles.tile([128, NT * NW], F32)
    smS = singles.tile([128, NT], F32)      # rowsum of mask (npos + 1)
    ratioS = singles.tile([128, NT], F32)   # pos_sum / (allsum - eself)
    hpS = singles.tile([128, NT], F32)      # has-positives per row
    pmE2 = singles.tile([128, NT], F32)     # pos_max (E-space, clamped)
    msumS = singles.tile([128, NT], F32)    # rowsum of mask*E (pos_sum + eself)
    top8b = singles.tile([128, NT * 8], F32)  # band top-8: [0]=eself, [1]=pos_max
    top8s = singles.tile([128, NT * 8], BF16)
    outsb = singles.tile([128, 32], F32)

    nc.gpsimd.memset(ones_col[:], 1.0)
    nc.gpsimd.memset(magic[:], 0x5F3759DF)
    nc.gpsimd.memset(one_i[:], 1)

    # ---- input DMAs ----
    et = [etpool.tile([128, B], BF16, name=f"et{k}") for k in range(KT)]
    etn = [etpool.tile([128, B], BF16, name=f"etn{k}") for k in range(KT)]
    labrow = singles.tile([1, LABW], F16)
    for c in range(NW):
        for k in range(KT):
            nc.sync.dma_start(
                et[k][:, c * W:(c + 1) * W],
                et_d[k * 128:(k + 1) * 128, c * W:(c + 1) * W],
            )

    # ---- column norms + normalize, chunk-pipelined ----
    # sq = et*et (bf16, DVE 2x); n2[m] = sum_k sq[k,m] via single-column
    # matmuls with sq as the STATIONARY operand -> n2 lands as [128, 64]
    # in PSUM directly (n2[128g+p] at [p, g]); inv computed per chunk on
    # the fat layout, DMA-rearranged to a [1,B] row for the GPSIMD
    # broadcast, then etn = et * inv (DVE tt, bf16 2x).
    G = W // 128  # column groups per chunk (16)
    # shares the "mm" slot rotation (slot 0); freed before sim mm t0w1 needs it
    n2ps = psmm.tile([128, B // 128], F32, tag="mm", name="n2ps")
    inv_dram = dramp.tile([1, B], BF16)
    for c in range(NW):
        cs = slice(c * G, (c + 1) * G)
        sqs = []
        for k in range(KT):
            sq = sqpool.tile([128, W], BF16, tag=f"sq{k}", name=f"sq{k}_{c}")
            if k == 0:
                # ACT is idle during the preamble; all Squares precede the
                # first Exp so the table loads only once.
                nc.scalar.activation(
                    sq[:], et[k][:, c * W:(c + 1) * W], AF.Square
                )
            else:
                nc.vector.tensor_tensor(
                    out=sq[:],
                    in0=et[k][:, c * W:(c + 1) * W],
                    in1=et[k][:, c * W:(c + 1) * W],
                    op=ALU.mult,
                )
            sqs.append(sq)
        for gl in range(G):
            g = c * G + gl
            for k in range(KT):
                nc.tensor.matmul(
                    n2ps[:, g:g + 1],
                    sqs[k][:, gl * 128:(gl + 1) * 128],
                    ones_col[:],
                    start=(k == 0), stop=(k == KT - 1),
                )
        # inv = rsqrt(max(n2, 1e-24)) on this chunk's [128, 16] slice --
        # DVE-only (bit-trick seed + 2 Newton steps) so ACT never loads a
        # Sqrt table between the main-loop Exp activations.
        nc.vector.tensor_scalar(
            out=n2pt[:, cs], in0=n2ps[:, cs], scalar1=1e-24, scalar2=None,
            op0=ALU.max,
        )
        nc.vector.tensor_tensor(
            out=ish[:, cs], in0=n2pt[:, cs].bitcast(I32), in1=one_i[:, cs],
            op=ALU.logical_shift_right,
        )
        nc.vector.tensor_tensor(
            out=invpt[:, cs].bitcast(I32), in0=magic[:, cs], in1=ish[:, cs],
            op=ALU.subtract,
        )
        for _ in range(2):
            nc.vector.tensor_tensor(
                out=nt1[:, cs], in0=invpt[:, cs], in1=invpt[:, cs],
                op=ALU.mult,
            )
            nc.vector.scalar_tensor_tensor(
                out=nt2[:, cs], in0=n2pt[:, cs], scalar=-0.5, in1=nt1[:, cs],
                op0=ALU.mult, op1=ALU.mult,
            )
            nc.vector.tensor_scalar_add(nt2[:, cs], nt2[:, cs], 1.5)
            nc.vector.tensor_tensor(
                out=invpt[:, cs], in0=invpt[:, cs], in1=nt2[:, cs],
                op=ALU.mult,
            )
        nc.vector.tensor_copy(out=invptb[:, cs], in_=invpt[:, cs])
        nc.sync.dma_start(
            inv_dram[0, c * W:(c + 1) * W].rearrange("(t p) -> p t", p=128),
            invptb[:, cs],
        )
        nc.sync.dma_start(
            invrowb[0:1, c * W:(c + 1) * W],
            inv_dram[0:1, c * W:(c + 1) * W],
        )
        if c == 0:
            # per-row scale inv_i / T (rows 128..1152 live in chunk 0)
            nc.sync.dma_start(
                invrows8[:],
                inv_dram[0:1, 128:128 + RPC].rearrange(
                    "o (t p) -> o p t", p=128
                ),
            )
            nc.vector.tensor_scalar_mul(invrowsT[:], invrows8[:], INVT)
        invb = invbpool.tile([128, W], BF16, tag="invb", name=f"invb{c}")
        nc.gpsimd.partition_broadcast(invb[:], invrowb[0:1, c * W:(c + 1) * W])
        for k in range(KT):
            nc.vector.tensor_tensor(
                out=etn[k][:, c * W:(c + 1) * W],
                in0=et[k][:, c * W:(c + 1) * W],
                in1=invb[:],
                op=ALU.mult,
            )
        if c == 0:
            nc.sync.dma_start(eye[:], eye_d[:, :])
            nc.sync.dma_start(labrow[:], lab_d[0:1, :])
            nc.sync.dma_start(
                lab_rows16[:],
                lab_d[0:1, 128:128 + RPC].rearrange("o (t p) -> o p t", p=128),
            )
            nc.vector.tensor_copy(out=lab_rows[:], in_=lab_rows16[:])
            nc.gpsimd.partition_broadcast(lab_bc[:], labrow[0:1, :])

    # ---- main loop over row tiles ----
    def alloc_tiles(t):
        # E chunk 0 is fp32 (the label band lives there): the accumulated
        # row sum then matches the stored values bit-exactly, so eself/
        # pos_sum subtract cleanly. Chunks 1-3 are bf16 for the 2x tree.
        E0f = epool.tile([128, W], F32, tag="E0f", name=f"E0f_{t}")
        E = epool.tile([128, B - W], BF16, tag="E", name=f"E_{t}")
        return E0f, E

    def emit_mm_exp(t, w, E0f, E):
        lo = 128 + t * 128
        ps = psmm.tile([128, W], F32, tag="mm", name=f"mm{t}_{w}")
        for j in range(W // MM):
            c0 = w * W + j * MM
            for k in range(KT):
                nc.tensor.matmul(
                    ps[:, j * MM:(j + 1) * MM],
                    et[k][:, lo:lo + 128],
                    etn[k][:, c0:c0 + MM],
                    start=(k == 0), stop=(k == KT - 1),
                )
        dst = E0f[:] if w == 0 else E[:, (w - 1) * W:w * W]
        nc.scalar.activation(
            dst, ps[:], AF.Exp,
            scale=invrowsT[:, t:t + 1],
            accum_out=asum[:, t * NW + w:t * NW + w + 1],
        )

    def emit_band_tree(t, E0f, E):
        bl = t * 128
        # band stats: positives live in cols [bl, bl+BAND)
        mask = bandpool.tile([128, BAND], BF16, tag="mask", name=f"mask{t}")
        scrm = bandpool.tile([128, BAND], F32, tag="scrm", name=f"scrm{t}")
        nc.vector.tensor_scalar(
            out=mask[:], in0=lab_bc[:, bl:bl + BAND],
            scalar1=lab_rows[:, t:t + 1], scalar2=0.0, op0=ALU.is_equal,
            op1=ALU.add, accum_out=smS[:, t:t + 1],
        )
        # scrm = mask * E0f: top-1 = eself (sim_ii==1 dominates the band),
        # top-2 = pos_max; fused accum = pos_sum + eself
        nc.vector.scalar_tensor_tensor(
            out=scrm[:], in0=mask[:], scalar=1.0, in1=E0f[:, bl:bl + BAND],
            op0=ALU.mult, op1=ALU.mult,
            accum_out=msumS[:, t:t + 1],
        )
        nc.vector.max(top8b[:, t * 8:(t + 1) * 8], scrm[:])
        # mask same-class (incl self) out for the negatives top-k
        nc.vector.scalar_tensor_tensor(
            out=E0f[:, bl:bl + BAND], in0=mask[:], scalar=NEG_BIG,
            in1=E0f[:, bl:bl + BAND], op0=ALU.mult, op1=ALU.add,
        )
        # negatives top-8 over disjoint window maxes; the chunk-0/1 side
        # folds to 512 early so only the chunk-2/3 side + merge runs after
        # the last exp of the tile.
        m01 = treepool.tile([128, W], BF16, tag="m01", name=f"m01_{t}")
        m23 = treepool.tile([128, W], BF16, tag="m23", name=f"m23_{t}")
        m2 = treepool.tile([128, W // 2], BF16, tag="m2", name=f"m2_{t}")
        m3 = treepool.tile([128, W // 4], BF16, tag="m3", name=f"m3_{t}")
        m2b = treepool.tile([128, W // 2], BF16, tag="m2b", name=f"m2b_{t}")
        m3b = treepool.tile([128, W // 4], BF16, tag="m3b", name=f"m3b_{t}")
        nc.vector.tensor_tensor(
            out=m01[:], in0=E0f[:], in1=E[:, 0:W], op=ALU.max
        )
        nc.vector.tensor_tensor(
            out=m2[:], in0=m01[:, 0:W // 2], in1=m01[:, W // 2:W], op=ALU.max
        )
        nc.vector.tensor_tensor(
            out=m3[:], in0=m2[:, 0:W // 4], in1=m2[:, W // 4:W // 2],
            op=ALU.max,
        )
        nc.vector.tensor_tensor(
            out=m23[:], in0=E[:, W:2 * W], in1=E[:, 2 * W:3 * W], op=ALU.max
        )
        nc.vector.tensor_tensor(
            out=m2b[:], in0=m23[:, 0:W // 2], in1=m23[:, W // 2:W],
            op=ALU.max,
        )
        nc.vector.tensor_tensor(
            out=m3b[:], in0=m2b[:, 0:W // 4], in1=m2b[:, W // 4:W // 2],
            op=ALU.max,
        )
        nc.vector.tensor_tensor(
            out=m3[:], in0=m3[:], in1=m3b[:], op=ALU.max
        )
        nc.vector.max(top8s[:, t * 8:(t + 1) * 8], m3[:])
        # per-tile loss prefix on [128,1] slices (rides the DVE slack):
        # ratio = (msum - eself) / (allsum - eself + 1e-10), hp, pos_max
        al = bandpool.tile([128, 1], F32, tag="al", name=f"al{t}")
        rp1 = bandpool.tile([128, 1], F32, tag="rp1", name=f"rp1{t}")
        ps1 = bandpool.tile([128, 1], F32, tag="ps1", name=f"ps1{t}")
        eself1 = top8b[:, t * 8:t * 8 + 1]
        nc.vector.tensor_reduce(
            out=al[:], in_=asum[:, t * NW:(t + 1) * NW], axis=AX.X,
            op=ALU.add,
        )
        nc.vector.tensor_tensor(
            out=al[:], in0=al[:], in1=eself1, op=ALU.subtract
        )
        nc.vector.tensor_scalar_add(al[:], al[:], 1e-10)
        nc.vector.reciprocal(rp1[:], al[:])
        nc.vector.tensor_tensor(
            out=ps1[:], in0=msumS[:, t:t + 1], in1=eself1, op=ALU.subtract
        )
        nc.vector.scalar_tensor_tensor(
            out=ratioS[:, t:t + 1], in0=ps1[:], scalar=1.0, in1=rp1[:],
            op0=ALU.mult, op1=ALU.mult,
        )
        nc.vector.tensor_scalar_add(
            ratioS[:, t:t + 1], ratioS[:, t:t + 1], 1e-10
        )
        nc.vector.tensor_scalar(
            out=hpS[:, t:t + 1], in0=smS[:, t:t + 1], scalar1=1.5,
            scalar2=None, op0=ALU.is_ge,
        )
        nc.vector.tensor_scalar_max(
            pmE2[:, t:t + 1], top8b[:, t * 8 + 1:t * 8 + 2], 1e-30
        )

    # Tiles 0 and 1 interleave chunk-wise: each etn chunk arriving from the
    # preamble feeds two tiles of exp work, keeping ACT busy during warmup.
    E0f_a, E_a = alloc_tiles(0)
    E0f_b, E_b = alloc_tiles(1)
    for w in range(NW):
        emit_mm_exp(0, w, E0f_a, E_a)
        emit_mm_exp(1, w, E0f_b, E_b)
    emit_band_tree(0, E0f_a, E_a)
    emit_band_tree(1, E0f_b, E_b)
    for t in range(2, NT):
        E0f, E = alloc_tiles(t)
        for w in range(NW):
            emit_mm_exp(t, w, E0f, E)
        emit_band_tree(t, E0f, E)

    # ---- epilogue: only the Ln's and the loss combines remain ----
    ep = ctx.enter_context(tc.tile_pool(name="ep", bufs=1))
    Lb = ep.tile([128, NT], F32)
    pmx = ep.tile([128, NT], F32)
    l3 = ep.tile([128, NT * 3], F32)
    s123 = ep.tile([128, NT], F32)
    u = ep.tile([128, NT], F32)
    v = ep.tile([128, NT], F32)

    nc.scalar.activation(Lb[:], ratioS[:], AF.Ln)
    nc.scalar.activation(pmx[:], pmE2[:], AF.Ln)
    # top-3 negative sims (ln units)
    nc.scalar.activation(
        l3[:].rearrange("p (t k) -> p t k", k=3),
        top8s[:].rearrange("p (t k) -> p t k", k=8)[:, :, 0:3],
        AF.Ln,
    )
    nc.vector.tensor_reduce(
        out=s123[:], in_=l3[:].rearrange("p (t k) -> p t k", k=3),
        axis=AX.X, op=ALU.add,
    )
    # hard: h = relu(s123/3 - pmx + MARGIN) * hp
    nc.vector.scalar_tensor_tensor(
        out=u[:], in0=s123[:], scalar=1.0 / 3.0, in1=pmx[:],
        op0=ALU.mult, op1=ALU.subtract,
    )
    nc.vector.tensor_scalar(
        out=v[:], in0=u[:], scalar1=MARGIN, scalar2=0.0,
        op0=ALU.add, op1=ALU.max,
    )
    nc.vector.tensor_tensor(
        out=outsb[:, 16:24], in0=v[:], in1=hpS[:], op=ALU.mult
    )
    # margin: m = relu(s1 - pmx + MARGIN) * hp
    nc.vector.scalar_tensor_tensor(
        out=u[:], in0=l3[:].rearrange("p (t k) -> p t k", k=3)[:, :, 0],
        scalar=1.0, in1=pmx[:], op0=ALU.mult, op1=ALU.subtract,
    )
    nc.vector.tensor_scalar(
        out=v[:], in0=u[:], scalar1=MARGIN, scalar2=0.0,
        op0=ALU.add, op1=ALU.max,
    )
    nc.vector.tensor_tensor(
        out=outsb[:, 24:32], in0=v[:], in1=hpS[:], op=ALU.mult
    )
    # basic: -ln(ratio) * hp
    nc.vector.scalar_tensor_tensor(
        out=outsb[:, 0:8], in0=Lb[:], scalar=-1.0, in1=hpS[:],
        op0=ALU.mult, op1=ALU.mult,
    )
    nc.vector.tensor_copy(out=outsb[:, 8:16], in_=hpS[:])

    nc.sync.dma_start(out_d[:, :], outsb[:])


def _prep_inputs(embeddings, labels):
    e = np.ascontiguousarray(np.asarray(embeddings), dtype=np.float32)
    lab = np.asarray(labels)
    assert e.shape == (B, D) and lab.shape == (B,)
    perm = np.argsort(lab, kind="stable")
    e_s = e[perm]
    lab_s = lab[perm].astype(np.float16)
    counts = np.bincount(lab[perm].astype(np.int64))
    assert counts.max() <= 128, f"class size {counts.max()} > band margin"

    eye = np.zeros((128, BAND), dtype=ml_dtypes.bfloat16)
    eye[np.arange(128), 128 + np.arange(128)] = 1.0

    in_maps = []
    for c in range(NC):
        s = (c * RPC - 128) % B
        er = np.concatenate([e_s[s:], e_s[:s]], axis=0)
        lr = np.concatenate([lab_s[s:], lab_s[:s]])
        in_maps.append(
            {
                "et": np.ascontiguousarray(er.T).astype(ml_dtypes.bfloat16),
                "labf": np.ascontiguousarray(lr[None, :LABW]),
                "eye": eye,
            }
        )
    return in_maps


def _combine(results):
    SA = np.float32(0.0)
    SB = np.float32(0.0)
    SC = np.float32(0.0)
    SD = np.float32(0.0)
    for r in results:
        o = r["out"].astype(np.float32)
        SA += o[:, 0:8].sum(dtype=np.float32)
        SB += o[:, 8:16].sum(dtype=np.float32)
        SC += o[:, 16:24].sum(dtype=np.float32)
        SD += o[:, 24:32].sum(dtype=np.float32)
    nhp = max(SB, np.float32(1.0))
    basic = SA / nhp
    hard = SC / nhp
    margin = SD / nhp if SB > 0 else np.float32(0.0)
    total = basic + np.float32(0.5) * hard + np.float32(0.1) * margin
    return np.asarray(total, dtype=np.float32)


def kernel(embeddings, labels):
    in_maps = _prep_inputs(embeddings, labels)
    nc = _build_program()
    res = run_bass_kernel_spmd(nc, in_maps, core_ids=list(range(NC)))
    return _combine(res.results)


# revision 56
# speedup vs baseline: 1.6876x; 1.0021x over previous
"""EnhancedContrastiveLoss on 8 Trainium2 NeuronCores (Bass/Tile).

Asymmetric normalization + bf16 datapath, tuned so the scalar engine's
exp stream is the only saturated resource in steady state.

Host side (layout only): sort samples by label; shard 1024 rows/core with
a per-core column rotation so every core sees its rows' class neighborhood
at the same local columns (SPMD-constant addressing); ship et=[D,B] bf16,
labels fp16, eye bf16.

Device side (per core):
  * column norms: sq = et*et (half on ACT Square, half on DVE bf16 tt);
    n2 = per-128-column-group single-column matmuls with sq STATIONARY,
    landing n2 as [128,64] in PSUM directly; inv = rsqrt(max(n2,1e-24))
    via DVE-only bit-trick + 2 Newton steps (no ACT Sqrt table load);
    DMA-rearranged to a [1,B] row and partition-broadcast on GPSIMD.
  * only the RHS matmul operand is normalized (etn = et*inv); rows stay
    raw and exp applies a per-row scale inv_i/T on ACT:
    exp(raw_ij * inv_i / T) == exp(sim_ij / T) exactly.
  * sim row-tiles as 4x [128,2048] PSUM chunks (bf16 matmuls, 512-wide);
    ACT exp with fused row-sum accum. Chunk 0 (which contains the class
    band) is written fp32 so the accumulated row sums match the stored
    values bit-exactly; chunks 1-3 are bf16 for the 2x DVE max tree.
  * band stats in one pass: scrm = mask*E0f; its InstMax top-8 yields
    eself (top-1: sim_ii==1 dominates the band) and pos_max (top-2);
    its fused accum yields pos_sum + eself.
  * negatives top-8: disjoint-window bf16 tensor_tensor max tree (DVE 2x)
    folded to 512 candidates, then InstMax; the chunk-0/1 side folds
    early so only the chunk-2/3 side runs after the tile's last exp.
Host side: combine 8 cores' [128,32] partials into the 3 scalar losses.

Note: InstTensorTensorReduce aborts on this runtime (even all-fp32), and
generic tensor ops (stt/copy/reduce) fail the Pool-engine ISA check --
band reductions use scalar_tensor_tensor accum_out on DVE instead, and
GPSIMD only runs memset/partition_broadcast.
"""

import numpy as np
import ml_dtypes
from contextlib import ExitStack

import concourse.bass as bass
import concourse.mybir as mybir
from concourse import bacc, tile
from concourse.bass_utils import run_bass_kernel_spmd

F32 = mybir.dt.float32
BF16 = mybir.dt.bfloat16
F16 = mybir.dt.float16
I32 = mybir.dt.int32
AF = mybir.ActivationFunctionType
ALU = mybir.AluOpType
AX = mybir.AxisListType

B = 8192
D = 256
NC = 8
RPC = B // NC          # rows per core (1024)
NT = RPC // 128        # row tiles per core (8)
KT = D // 128          # K tiles (2)
BAND = 384
LABW = NT * 128 + BAND - 128   # 1280: label window needed on device
W = 2048               # PSUM chunk width (4 banks)
NW = B // W            # 4
MM = 512               # matmul moving-dim max
TEMP = 0.07
MARGIN = 0.2
INVT = 1.0 / TEMP
NEG_BIG = -1.0e30

_CACHE = {}


def _build_program():
    if "nc" in _CACHE:
        return _CACHE["nc"]
    nc = bacc.Bacc(
        "TRN2", target_bir_lowering=False, debug=False, num_devices=NC
    )
    et_d = nc.dram_tensor("et", [D, B], BF16, kind="ExternalInput").ap()
    lab_d = nc.dram_tensor("labf", [1, LABW], F16, kind="ExternalInput").ap()
    eye_d = nc.dram_tensor("eye", [128, BAND], BF16, kind="ExternalInput").ap()
    out_d = nc.dram_tensor("out", [128, 32], F32, kind="ExternalOutput").ap()

    with tile.TileContext(nc) as tc:
        with ExitStack() as ctx:
            _body(ctx, tc, et_d, lab_d, eye_d, out_d)

    nc.finalize()
    _CACHE["nc"] = nc
    return nc


def _body(ctx, tc, et_d, lab_d, eye_d, out_d):
    nc = tc.nc

    singles = ctx.enter_context(tc.tile_pool(name="singles", bufs=1))
    etpool = ctx.enter_context(tc.tile_pool(name="et", bufs=1))
    epool = ctx.enter_context(tc.tile_pool(name="E", bufs=3))
    sqpool = ctx.enter_context(tc.tile_pool(name="sq", bufs=2))
    invbpool = ctx.enter_context(tc.tile_pool(name="invb", bufs=2))
    bandpool = ctx.enter_context(tc.tile_pool(name="band", bufs=2))
    treepool = ctx.enter_context(tc.tile_pool(name="tree", bufs=2))
    psmm = ctx.enter_context(tc.tile_pool(name="psmm", bufs=2, space="PSUM"))
    dramp = ctx.enter_context(tc.tile_pool(name="dramp", bufs=1, space="DRAM"))

    # ---- persistent tiles ----
    ones_col = singles.tile([128, 1], BF16)
    lab_bc = singles.tile([128, LABW], F16)
    lab_rows16 = singles.tile([128, NT], F16)
    lab_rows = singles.tile([128, NT], F32)
    eye = singles.tile([128, BAND], BF16)
    n2pt = singles.tile([128, B // 128], F32)     # [128, 64]
    invpt = singles.tile([128, B // 128], F32)
    invptb = singles.tile([128, B // 128], BF16)
    ish = singles.tile([128, B // 128], I32)
    magic = singles.tile([128, B // 128], I32)
    one_i = singles.tile([128, B // 128], I32)
    nt1 = singles.tile([128, B // 128], F32)
    nt2 = singles.tile([128, B // 128], F32)
    invrowb = singles.tile([1, B], BF16)          # bcast source, partition 0
    invrows8 = singles.tile([128, NT], BF16)
    invrowsT = singles.tile([128, NT], F32)       # inv_i / T per row tile
    asum = singles.tile([128, NT * NW], F32)
    smS = singles.tile([128, NT], F32)      # rowsum of mask (npos + 1)
    ratioS = singles.tile([128, NT], F32)   # pos_sum / (allsum - eself)
    hpS = singles.tile([128, NT], F32)      # has-positives per row
    pmE2 = singles.tile([128, NT], F32)     # pos_max (E-space, clamped)
    msumS = singles.tile([128, NT], F32)    # rowsum of mask*E (pos_sum + eself)
    top8b = singles.tile([128, NT * 8], F32)  # band top-8: [0]=eself, [1]=pos_max
    top8s = singles.tile([128, NT * 8], BF16)
    outsb = singles.tile([128, 32], F32)

    nc.gpsimd.memset(ones_col[:], 1.0)
    nc.gpsimd.memset(magic[:], 0x5F3759DF)
    nc.gpsimd.memset(one_i[:], 1)

    # ---- input DMAs ----
    et = [etpool.tile([128, B], BF16, name=f"et{k}") for k in range(KT)]
    etn = [etpool.tile([128, B], BF16, name=f"etn{k}") for k in range(KT)]
    labrow = singles.tile([1, LABW], F16)
    for c in range(NW):
        for k in range(KT):
            nc.sync.dma_start(
                et[k][:, c * W:(c + 1) * W],
                et_d[k * 128:(k + 1) * 128, c * W:(c + 1) * W],
            )

    # ---- column norms + normalize, chunk-pipelined ----
    # sq = et*et (bf16, DVE 2x); n2[m] = sum_k sq[k,m] via single-column
    # matmuls with sq as the STATIONARY operand -> n2 lands as [128, 64]
    # in PSUM directly (n2[128g+p] at [p, g]); inv computed per chunk on
    # the fat layout, DMA-rearranged to a [1,B] row for the GPSIMD
    # broadcast, then etn = et * inv (DVE tt, bf16 2x).
    G = W // 128  # column groups per chunk (16)
    # shares the "mm" slot rotation (slot 0); freed before sim mm t0w1 needs it
    n2ps = psmm.tile([128, B // 128], F32, tag="mm", name="n2ps")
    inv_dram = dramp.tile([1, B], BF16)
    for c in range(NW):
        cs = slice(c * G, (c + 1) * G)
        sqs = []
        for k in range(KT):
            sq = sqpool.tile([128, W], BF16, tag=f"sq{k}", name=f"sq{k}_{c}")
            if k == 0:
                # ACT is idle during the preamble; all Squares precede the
                # first Exp so the table loads only once.
                nc.scalar.activation(
                    sq[:], et[k][:, c * W:(c + 1) * W], AF.Square
                )
            else:
                nc.vector.tensor_tensor(
                    out=sq[:],
                    in0=et[k][:, c * W:(c + 1) * W],
                    in1=et[k][:, c * W:(c + 1) * W],
                    op=ALU.mult,
                )
            sqs.append(sq)
        for gl in range(G):
            g = c * G + gl
            for k in range(KT):
                nc.tensor.matmul(
                    n2ps[:, g:g + 1],
                    sqs[k][:, gl * 128:(gl + 1) * 128],
                    ones_col[:],
                    start=(k == 0), stop=(k == KT - 1),
                )
        # inv = rsqrt(max(n2, 1e-24)) on this chunk's [128, 16] slice --
        # DVE-only (bit-trick seed + 2 Newton steps) so ACT never loads a
        # Sqrt table between the main-loop Exp activations.
        nc.vector.tensor_scalar(
            out=n2pt[:, cs], in0=n2ps[:, cs], scalar1=1e-24, scalar2=None,
            op0=ALU.max,
        )
        nc.vector.tensor_tensor(
            out=ish[:, cs], in0=n2pt[:, cs].bitcast(I32), in1=one_i[:, cs],
            op=ALU.logical_shift_right,
        )
        nc.vector.tensor_tensor(
            out=invpt[:, cs].bitcast(I32), in0=magic[:, cs], in1=ish[:, cs],
            op=ALU.subtract,
        )
        for _ in range(2):
            nc.vector.tensor_tensor(
                out=nt1[:, cs], in0=invpt[:, cs], in1=invpt[:, cs],
                op=ALU.mult,
            )
            nc.vector.scalar_tensor_tensor(
                out=nt2[:, cs], in0=n2pt[:, cs], scalar=-0.5, in1=nt1[:, cs],
                op0=ALU.mult, op1=ALU.mult,
            )
            nc.vector.tensor_scalar_add(nt2[:, cs], nt2[:, cs], 1.5)
            nc.vector.tensor_tensor(
                out=invpt[:, cs], in0=invpt[:, cs], in1=nt2[:, cs],
                op=ALU.mult,
            )
        nc.vector.tensor_copy(out=invptb[:, cs], in_=invpt[:, cs])
        nc.sync.dma_start(
            inv_dram[0, c * W:(c + 1) * W].rearrange("(t p) -> p t", p=128),
            invptb[:, cs],
        )
        nc.sync.dma_start(
            invrowb[0:1, c * W:(c + 1) * W],
            inv_dram[0:1, c * W:(c + 1) * W],
        )
        if c == 0:
            # per-row scale inv_i / T (rows 128..1152 live in chunk 0)
            nc.sync.dma_start(
                invrows8[:],
                inv_dram[0:1, 128:128 + RPC].rearrange(
                    "o (t p) -> o p t", p=128
                ),
            )
            nc.vector.tensor_scalar_mul(invrowsT[:], invrows8[:], INVT)
        invb = invbpool.tile([128, W], BF16, tag="invb", name=f"invb{c}")
        nc.gpsimd.partition_broadcast(invb[:], invrowb[0:1, c * W:(c + 1) * W])
        for k in range(KT):
            nc.vector.tensor_tensor(
                out=etn[k][:, c * W:(c + 1) * W],
                in0=et[k][:, c * W:(c + 1) * W],
                in1=invb[:],
                op=ALU.mult,
            )
        if c == 0:
            nc.sync.dma_start(eye[:], eye_d[:, :])
            nc.sync.dma_start(labrow[:], lab_d[0:1, :])
            nc.sync.dma_start(
                lab_rows16[:],
                lab_d[0:1, 128:128 + RPC].rearrange("o (t p) -> o p t", p=128),
            )
            nc.vector.tensor_copy(out=lab_rows[:], in_=lab_rows16[:])
    # lab broadcast AFTER the inv broadcasts: it is not needed until the
    # first band stats (~20us), while every inv bcast gates an etn chunk
    nc.gpsimd.partition_broadcast(lab_bc[:], labrow[0:1, :])

    # ---- main loop over row tiles ----
    def alloc_tiles(t):
        # E chunk 0 is fp32 (the label band lives there): the accumulated
        # row sum then matches the stored values bit-exactly, so eself/
        # pos_sum subtract cleanly. Chunks 1-3 are bf16 for the 2x tree.
        E0f = epool.tile([128, W], F32, tag="E0f", name=f"E0f_{t}")
        E = epool.tile([128, B - W], BF16, tag="E", name=f"E_{t}")
        return E0f, E

    def emit_mm_exp(t, w, E0f, E):
        lo = 128 + t * 128
        ps = psmm.tile([128, W], F32, tag="mm", name=f"mm{t}_{w}")
        for j in range(W // MM):
            c0 = w * W + j * MM
            for k in range(KT):
                nc.tensor.matmul(
                    ps[:, j * MM:(j + 1) * MM],
                    et[k][:, lo:lo + 128],
                    etn[k][:, c0:c0 + MM],
                    start=(k == 0), stop=(k == KT - 1),
                )
        dst = E0f[:] if w == 0 else E[:, (w - 1) * W:w * W]
        nc.scalar.activation(
            dst, ps[:], AF.Exp,
            scale=invrowsT[:, t:t + 1],
            accum_out=asum[:, t * NW + w:t * NW + w + 1],
        )

    def emit_band_tree(t, E0f, E):
        bl = t * 128
        # band stats: positives live in cols [bl, bl+BAND)
        mask = bandpool.tile([128, BAND], BF16, tag="mask", name=f"mask{t}")
        scrm = bandpool.tile([128, BAND], F32, tag="scrm", name=f"scrm{t}")
        nc.vector.tensor_scalar(
            out=mask[:], in0=lab_bc[:, bl:bl + BAND],
            scalar1=lab_rows[:, t:t + 1], scalar2=0.0, op0=ALU.is_equal,
            op1=ALU.add, accum_out=smS[:, t:t + 1],
        )
        # scrm = mask * E0f: top-1 = eself (sim_ii==1 dominates the band),
        # top-2 = pos_max; fused accum = pos_sum + eself
        nc.vector.scalar_tensor_tensor(
            out=scrm[:], in0=mask[:], scalar=1.0, in1=E0f[:, bl:bl + BAND],
            op0=ALU.mult, op1=ALU.mult,
            accum_out=msumS[:, t:t + 1],
        )
        nc.vector.max(top8b[:, t * 8:(t + 1) * 8], scrm[:])
        # mask same-class (incl self) out for the negatives top-k
        nc.vector.scalar_tensor_tensor(
            out=E0f[:, bl:bl + BAND], in0=mask[:], scalar=NEG_BIG,
            in1=E0f[:, bl:bl + BAND], op0=ALU.mult, op1=ALU.add,
        )
        # negatives top-8 over disjoint window maxes; the chunk-0/1 side
        # folds to 512 early so only the chunk-2/3 side + merge runs after
        # the last exp of the tile.
        m01 = treepool.tile([128, W], BF16, tag="m01", name=f"m01_{t}")
        m23 = treepool.tile([128, W], BF16, tag="m23", name=f"m23_{t}")
        m2 = treepool.tile([128, W // 2], BF16, tag="m2", name=f"m2_{t}")
        m3 = treepool.tile([128, W // 4], BF16, tag="m3", name=f"m3_{t}")
        m2b = treepool.tile([128, W // 2], BF16, tag="m2b", name=f"m2b_{t}")
        m3b = treepool.tile([128, W // 4], BF16, tag="m3b", name=f"m3b_{t}")
        nc.vector.tensor_tensor(
            out=m01[:], in0=E0f[:], in1=E[:, 0:W], op=ALU.max
        )
        nc.vector.tensor_tensor(
            out=m2[:], in0=m01[:, 0:W // 2], in1=m01[:, W // 2:W], op=ALU.max
        )
        nc.vector.tensor_tensor(
            out=m3[:], in0=m2[:, 0:W // 4], in1=m2[:, W // 4:W // 2],
            op=ALU.max,
        )
        nc.vector.tensor_tensor(
            out=m23[:], in0=E[:, W:2 * W], in1=E[:, 2 * W:3 * W], op=ALU.max
        )
        nc.vector.tensor_tensor(
            out=m2b[:], in0=m23[:, 0:W // 2], in1=m23[:, W // 2:W],
            op=ALU.max,
        )
        nc.vector.tensor_tensor(
            out=m3b[:], in0=m2b[:, 0:W // 4], in1=m2b[:, W // 4:W // 2],
            op=ALU.max,
        )
        nc.vector.tensor_tensor(
            out=m3[:], in0=m3[:], in1=m3b[:], op=ALU.max
        )
        nc.vector.max(top8s[:, t * 8:(t + 1) * 8], m3[:])
        # per-tile loss prefix on [128,1] slices (rides the DVE slack):
        # ratio = (msum - eself) / (allsum - eself + 1e-10), hp, pos_max
        al = bandpool.tile([128, 1], F32, tag="al", name=f"al{t}")
        rp1 = bandpool.tile([128, 1], F32, tag="rp1", name=f"rp1{t}")
        ps1 = bandpool.tile([128, 1], F32, tag="ps1", name=f"ps1{t}")
        eself1 = top8b[:, t * 8:t * 8 + 1]
        nc.vector.tensor_reduce(
            out=al[:], in_=asum[:, t * NW:(t + 1) * NW], axis=AX.X,
            op=ALU.add,
        )
        nc.vector.tensor_tensor(
            out=al[:], in0=al[:], in1=eself1, op=ALU.subtract
        )
        nc.vector.tensor_scalar_add(al[:], al[:], 1e-10)
        nc.vector.reciprocal(rp1[:], al[:])
        nc.vector.tensor_tensor(
            out=ps1[:], in0=msumS[:, t:t + 1], in1=eself1, op=ALU.subtract
        )
        nc.vector.scalar_tensor_tensor(
            out=ratioS[:, t:t + 1], in0=ps1[:], scalar=1.0, in1=rp1[:],
            op0=ALU.mult, op1=ALU.mult,
        )
        nc.vector.tensor_scalar_add(
            ratioS[:, t:t + 1], ratioS[:, t:t + 1], 1e-10
        )
        nc.vector.tensor_scalar(
            out=hpS[:, t:t + 1], in0=smS[:, t:t + 1], scalar1=1.5,
            scalar2=None, op0=ALU.is_ge,
        )
        nc.vector.tensor_scalar_max(
            pmE2[:, t:t + 1], top8b[:, t * 8 + 1:t * 8 + 2], 1e-30
        )

    # Tiles 0 and 1 interleave chunk-wise: each etn chunk arriving from the
    # preamble feeds two tiles of exp work, keeping ACT busy during warmup.
    E0f_a, E_a = alloc_tiles(0)
    E0f_b, E_b = alloc_tiles(1)
    for t, w in [(0, 0), (0, 1), (1, 0), (0, 2), (1, 1), (0, 3), (1, 2),
                 (1, 3)]:
        emit_mm_exp(t, w, E0f_a if t == 0 else E0f_b,
                    E_a if t == 0 else E_b)
    emit_band_tree(0, E0f_a, E_a)
    emit_band_tree(1, E0f_b, E_b)
    for tp in range(2, NT, 2):
        E0f_c, E_c = alloc_tiles(tp)
        E0f_d, E_d = alloc_tiles(tp + 1)
        for t, w in [(tp, 0), (tp, 1), (tp + 1, 0), (tp, 2), (tp + 1, 1),
                     (tp, 3), (tp + 1, 2), (tp + 1, 3)]:
            emit_mm_exp(t, w, E0f_c if t == tp else E0f_d,
                        E_c if t == tp else E_d)
        emit_band_tree(tp, E0f_c, E_c)
        emit_band_tree(tp + 1, E0f_d, E_d)

    # ---- epilogue: only the Ln's and the loss combines remain ----
    ep = ctx.enter_context(tc.tile_pool(name="ep", bufs=1))
    Lb = ep.tile([128, NT], F32)
    pmx = ep.tile([128, NT], F32)
    l3 = ep.tile([128, NT * 3], F32)
    s123 = ep.tile([128, NT], F32)
    u = ep.tile([128, NT], F32)
    v = ep.tile([128, NT], F32)

    nc.scalar.activation(Lb[:], ratioS[:], AF.Ln)
    nc.scalar.activation(pmx[:], pmE2[:], AF.Ln)
    # top-3 negative sims (ln units)
    nc.scalar.activation(
        l3[:].rearrange("p (t k) -> p t k", k=3),
        top8s[:].rearrange("p (t k) -> p t k", k=8)[:, :, 0:3],
        AF.Ln,
    )
    nc.vector.tensor_reduce(
        out=s123[:], in_=l3[:].rearrange("p (t k) -> p t k", k=3),
        axis=AX.X, op=ALU.add,
    )
    # hard: h = relu(s123/3 - pmx + MARGIN) * hp
    nc.vector.scalar_tensor_tensor(
        out=u[:], in0=s123[:], scalar=1.0 / 3.0, in1=pmx[:],
        op0=ALU.mult, op1=ALU.subtract,
    )
    nc.vector.tensor_scalar(
        out=v[:], in0=u[:], scalar1=MARGIN, scalar2=0.0,
        op0=ALU.add, op1=ALU.max,
    )
    nc.vector.tensor_tensor(
        out=outsb[:, 16:24], in0=v[:], in1=hpS[:], op=ALU.mult
    )
    # margin: m = relu(s1 - pmx + MARGIN) * hp
    nc.vector.scalar_tensor_tensor(
        out=u[:], in0=l3[:].rearrange("p (t k) -> p t k", k=3)[:, :, 0],
        scalar=1.0, in1=pmx[:], op0=ALU.mult, op1=ALU.subtract,
    )
    nc.vector.tensor_scalar(
        out=v[:], in0=u[:], scalar1=MARGIN, scalar2=0.0,
        op0=ALU.add, op1=ALU.max,
    )
    nc.vector.tensor_tensor(
        out=outsb[:, 24:32], in0=v[:], in1=hpS[:], op=ALU.mult
    )
    # basic: -ln(ratio) * hp
    nc.vector.scalar_tensor_tensor(
        out=outsb[:, 0:8], in0=Lb[:], scalar=-1.0, in1=hpS[:],
        op0=ALU.mult, op1=ALU.mult,
    )
    nc.vector.tensor_copy(out=outsb[:, 8:16], in_=hpS[:])

    nc.sync.dma_start(out_d[:, :], outsb[:])


def _prep_inputs(embeddings, labels):
    e = np.ascontiguousarray(np.asarray(embeddings), dtype=np.float32)
    lab = np.asarray(labels)
    assert e.shape == (B, D) and lab.shape == (B,)
    perm = np.argsort(lab, kind="stable")
    e_s = e[perm]
    lab_s = lab[perm].astype(np.float16)
    counts = np.bincount(lab[perm].astype(np.int64))
    assert counts.max() <= 128, f"class size {counts.max()} > band margin"

    eye = np.zeros((128, BAND), dtype=ml_dtypes.bfloat16)
    eye[np.arange(128), 128 + np.arange(128)] = 1.0

    in_maps = []
    for c in range(NC):
        s = (c * RPC - 128) % B
        er = np.concatenate([e_s[s:], e_s[:s]], axis=0)
        lr = np.concatenate([lab_s[s:], lab_s[:s]])
        in_maps.append(
            {
                "et": np.ascontiguousarray(er.T).astype(ml_dtypes.bfloat16),
                "labf": np.ascontiguousarray(lr[None, :LABW]),
                "eye": eye,
            }
        )
    return in_maps


def _combine(results):
    SA = np.float32(0.0)
    SB = np.float32(0.0)
    SC = np.float32(0.0)
    SD = np.float32(0.0)
    for r in results:
        o = r["out"].astype(np.float32)
        SA += o[:, 0:8].sum(dtype=np.float32)
        SB += o[:, 8:16].sum(dtype=np.float32)
        SC += o[:, 16:24].sum(dtype=np.float32)
        SD += o[:, 24:32].sum(dtype=np.float32)
    nhp = max(SB, np.float32(1.0))
    basic = SA / nhp
    hard = SC / nhp
    margin = SD / nhp if SB > 0 else np.float32(0.0)
    total = basic + np.float32(0.5) * hard + np.float32(0.1) * margin
    return np.asarray(total, dtype=np.float32)


def kernel(embeddings, labels):
    in_maps = _prep_inputs(embeddings, labels)
    nc = _build_program()
    res = run_bass_kernel_spmd(nc, in_maps, core_ids=list(range(NC)))
    return _combine(res.results)
